# revision 10
# baseline (speedup 1.0000x reference)
"""Trainium2 Bass kernel: GatedRecurrentCell.

Math (per batch b):
    pa = x @ Wa^T + ba ; pi = x @ Wi^T + bi
    a  = sigmoid(gate) * 3**(-sigmoid(pa)) = exp(-ln3*sigmoid(pa) + ln(sigmoid(gate)))
    c  = sqrt(1-a^2) * silu(pi)
    h_t = a_t*h_{t-1} + c_t   (scan over time, h_{-1}=0);  out = h

Mapping: data-parallel over batch (8 cores, 1 batch each). On-chip layout is
channels-on-partitions / time-on-free-dim so the recurrence runs natively on
the DVE `tensor_tensor_scan` instruction. The host feeds pre-transposed
operand layouts (d-major x and W for the PE's contraction-on-partitions
matmul) and transposes the [I,S] per-core result back to [S,I] on the host.
"""

import functools
import os

import numpy as np

B, S, D, I = 8, 2048, 512, 2048
P = 128
NCORES = 8
LN3 = float(np.log(3.0))

# matmul input dtype: "f32r" (full-rate fp32 mode) or "f32" (4x slower, exact)
MM_MODE = os.environ.get("GRC_MM_MODE", "f32r")


def _build_nc(s, d, i, mm_mode=MM_MODE, ic_group=2, silu=True):
    import concourse.bacc as bacc
    import concourse.mybir as mybir
    import concourse.tile as tile
    from concourse.tile import add_dep_helper

    F32 = mybir.dt.float32
    F32R = mybir.dt.float32r
    AF = mybir.ActivationFunctionType
    ALU = mybir.AluOpType

    nd = d // P          # d chunks (contraction)
    ni = i // P          # i chunks (channel groups / partitions)
    ntc = s // 512       # time chunks for the matmul moving dim

    nc = bacc.Bacc("TRN2", target_bir_lowering=False, debug=False,
                   num_devices=NCORES)

    xT_d = nc.dram_tensor("xT", [d, s], F32, kind="ExternalInput").ap()
    waT_d = nc.dram_tensor("WaT", [ni, P, d], F32, kind="ExternalInput").ap()
    wiT_d = nc.dram_tensor("WiT", [ni, P, d], F32, kind="ExternalInput").ap()
    ba_d = nc.dram_tensor("baT", [P, ni], F32, kind="ExternalInput").ap()
    bi_d = nc.dram_tensor("biT", [P, ni], F32, kind="ExternalInput").ap()
    g_d = nc.dram_tensor("gateT", [P, ni], F32, kind="ExternalInput").ap()
    out_d = nc.dram_tensor("out", [i, s], F32, kind="ExternalOutput").ap()

    MMDT = F32R if mm_mode == "f32r" else F32

    def mm_ap(ap):
        return ap

    with tile.TileContext(nc) as tc:
        from contextlib import ExitStack

        with ExitStack() as ctx:
            const_pool = ctx.enter_context(tc.tile_pool(name="const", bufs=1))
            xt_pool = ctx.enter_context(tc.tile_pool(name="xt", bufs=1))
            wst_pool = ctx.enter_context(tc.tile_pool(name="wstream", bufs=1))
            ps_pool = ctx.enter_context(
                tc.tile_pool(name="mmpsum", bufs=1, space="PSUM"))
            chunk_pool = ctx.enter_context(tc.tile_pool(name="chunks", bufs=1))
            row_pool = ctx.enter_context(tc.tile_pool(name="rows", bufs=1))

            # ---- constants / per-channel vectors -------------------------
            ba_t = const_pool.tile([P, ni], F32, name="ba_t")
            nc.sync.dma_start(ba_t[:], ba_d[:])
            bi_t = const_pool.tile([P, ni], F32, name="bi_t")
            nc.sync.dma_start(bi_t[:], bi_d[:])
            g_t = const_pool.tile([P, ni], F32, name="g_t")
            nc.sync.dma_start(g_t[:], g_d[:])

            act_chain = []

            def act(out_ap, in_ap, func, **kw):
                inst = nc.scalar.activation(out_ap, in_ap, func, **kw)
                if act_chain:
                    add_dep_helper(inst.ins, act_chain[-1].ins,
                                   reason="act table phase order")
                act_chain.append(inst)
                return inst

            alpha_t = const_pool.tile([P, ni], F32, name="alpha_t")
            act(alpha_t[:], g_t[:], AF.Sigmoid)
            lna_t = const_pool.tile([P, ni], F32, name="lna_t")
            act(lna_t[:], alpha_t[:], AF.Ln)

            # ---- resident x^T tiles -------------------------------------
            xT_sb = []
            for k in range(nd):
                t_ = xt_pool.tile([P, s], MMDT, name=f"xT{k}", tag=f"xT{k}")
                nc.sync.dma_start(t_[:], xT_d[k * P:(k + 1) * P, :].bitcast(MMDT))
                xT_sb.append(t_)

            # ---- main loop: groups of `ic_group` channel chunks ---------
            ngroups = ni // ic_group
            for g in range(ngroups):
                ics = list(range(g * ic_group, (g + 1) * ic_group))

                # stream weights + GEMMs (PE) ------------------------------
                pa_ps, pi_ps = {}, {}
                for ic in ics:
                    wa_sb = wst_pool.tile([P, d], MMDT, name=f"wa{ic}",
                                          tag="wa", bufs=3)
                    nc.sync.dma_start(wa_sb[:], waT_d[ic].bitcast(MMDT))
                    wi_sb = wst_pool.tile([P, d], MMDT, name=f"wi{ic}",
                                          tag="wi", bufs=3)
                    nc.sync.dma_start(wi_sb[:], wiT_d[ic].bitcast(MMDT))
                    for t in range(ntc):
                        ps = ps_pool.tile([P, 512], F32, name=f"pa{ic}_{t}",
                                          tag="pa", bufs=3)
                        for k in range(nd):
                            nc.tensor.matmul(
                                ps[:],
                                mm_ap(wa_sb[:, k * P:(k + 1) * P]),
                                mm_ap(xT_sb[k][:, t * 512:(t + 1) * 512]),
                                start=(k == 0), stop=(k == nd - 1))
                        pa_ps[ic, t] = ps
                    for t in range(ntc):
                        ps = ps_pool.tile([P, 512], F32, name=f"pi{ic}_{t}",
                                          tag="pi", bufs=5)
                        for k in range(nd):
                            nc.tensor.matmul(
                                ps[:],
                                mm_ap(wi_sb[:, k * P:(k + 1) * P]),
                                mm_ap(xT_sb[k][:, t * 512:(t + 1) * 512]),
                                start=(k == 0), stop=(k == nd - 1))
                        pi_ps[ic, t] = ps

                # phase 1 (ACT: Sigmoid): s = sigmoid(pa + ba) ------------
                s_t = {}
                for ic in ics:
                    for t in range(ntc):
                        st = chunk_pool.tile([P, 512], F32, name=f"s{ic}_{t}",
                                             tag="s", bufs=4 * ic_group + 2)
                        act(st[:], pa_ps[ic, t][:], AF.Sigmoid,
                            bias=ba_t[:, ic:ic + 1])
                        s_t[ic, t] = st

                # phase 2 (ACT: Silu): w = silu(pi + bi) ------------------
                w_t = {}
                for ic in ics:
                    for t in range(ntc):
                        wt = chunk_pool.tile([P, 512], F32, name=f"w{ic}_{t}",
                                             tag="w", bufs=4 * ic_group + 2)
                        if silu:
                            act(wt[:], pi_ps[ic, t][:], AF.Silu,
                                bias=bi_t[:, ic:ic + 1])
                        else:
                            # sim-compatible fallback: sigmoid(pi+bi)*(pi+bi)
                            sg = chunk_pool.tile(
                                [P, 512], F32, name=f"sg{ic}_{t}",
                                tag="sg", bufs=3)
                            act(sg[:], pi_ps[ic, t][:], AF.Sigmoid,
                                bias=bi_t[:, ic:ic + 1])
                            pib = chunk_pool.tile(
                                [P, 512], F32, name=f"pib{ic}_{t}",
                                tag="pib", bufs=3)
                            act(pib[:], pi_ps[ic, t][:], AF.Identity,
                                bias=bi_t[:, ic:ic + 1])
                            nc.vector.tensor_mul(wt[:], sg[:], pib[:])
                        w_t[ic, t] = wt

                # phase 3 (ACT: Exp): a = exp(-ln3*s + ln(alpha)) ---------
                a_row = {}
                for ic in ics:
                    ar = row_pool.tile([P, s], F32, name=f"a{ic}", tag="a",
                                       bufs=ic_group + 1)
                    a_row[ic] = ar
                    for t in range(ntc):
                        act(ar[:, t * 512:(t + 1) * 512], s_t[ic, t][:],
                            AF.Exp, scale=-LN3, bias=lna_t[:, ic:ic + 1])

                # DVE: a2 = a*a (interleaves with phases 3/4) -------------
                a2_t = {}
                for ic in ics:
                    for t in range(ntc):
                        a2 = chunk_pool.tile([P, 512], F32, name=f"a2{ic}_{t}",
                                             tag="a2", bufs=4)
                        sl = a_row[ic][:, t * 512:(t + 1) * 512]
                        nc.vector.tensor_mul(a2[:], sl, sl)
                        a2_t[ic, t] = a2

                # phase 4 (ACT: Sqrt): q = sqrt(1 - a2); DVE: c = q*w -----
                c_row = {}
                for ic in ics:
                    cr = row_pool.tile([P, s], F32, name=f"c{ic}", tag="c",
                                       bufs=ic_group + 1)
                    c_row[ic] = cr
                    for t in range(ntc):
                        q = chunk_pool.tile([P, 512], F32, name=f"q{ic}_{t}",
                                            tag="q", bufs=4)
                        act(q[:], a2_t[ic, t][:], AF.Sqrt,
                            scale=-1.0, bias=1.0)
                        nc.vector.tensor_mul(
                            cr[:, t * 512:(t + 1) * 512], q[:], w_t[ic, t][:])

                # DVE scan + DMA out --------------------------------------
                for ic in ics:
                    h = row_pool.tile([P, s], F32, name=f"h{ic}", tag="h",
                                      bufs=3)
                    nc.vector.tensor_tensor_scan(
                        h[:], a_row[ic][:], c_row[ic][:], 0.0,
                        op0=ALU.mult, op1=ALU.add)
                    nc.sync.dma_start(out_d[ic * P:(ic + 1) * P, :], h[:])

    nc.compile()
    return nc


@functools.lru_cache(maxsize=2)
def _get_nc(s=S, d=D, i=I):
    return _build_nc(s, d, i)


LAST_RESULTS = None


def _prep_core_inputs(xb, WaT, WiT, baT, biT, gateT):
    return {"xT": np.ascontiguousarray(xb.T), "WaT": WaT, "WiT": WiT,
            "baT": baT, "biT": biT, "gateT": gateT}


def _prep_shared(Wa, ba, Wi, bi, gate, d, i):
    ni = i // P
    nd = d // P
    # WaT[ic, p, k*128+j] = Wa[ic*128+j, k*128+p]  (lhsT blocks, contiguous)
    WaT = np.ascontiguousarray(
        Wa.reshape(ni, P, nd, P).transpose(0, 3, 2, 1).reshape(ni, P, d))
    WiT = np.ascontiguousarray(
        Wi.reshape(ni, P, nd, P).transpose(0, 3, 2, 1).reshape(ni, P, d))
    baT = np.ascontiguousarray(ba.reshape(ni, P).T)
    biT = np.ascontiguousarray(bi.reshape(ni, P).T)
    gateT = np.ascontiguousarray(gate.reshape(ni, P).T)
    return WaT, WiT, baT, biT, gateT


def kernel(x, Wa, ba, Wi, bi, gate):
    global LAST_RESULTS
    from concourse.bass_utils import run_bass_kernel_spmd

    x = np.asarray(x, dtype=np.float32)
    b, s, d = x.shape
    i = Wa.shape[0]
    nc = _get_nc(s, d, i)

    WaT, WiT, baT, biT, gateT = _prep_shared(
        np.asarray(Wa, np.float32), np.asarray(ba, np.float32),
        np.asarray(Wi, np.float32), np.asarray(bi, np.float32),
        np.asarray(gate, np.float32), d, i)

    in_maps = [_prep_core_inputs(x[bb], WaT, WiT, baT, biT, gateT)
               for bb in range(b)]
    res = run_bass_kernel_spmd(nc, in_maps, list(range(b)))
    LAST_RESULTS = res
    out = np.stack([res.results[bb]["out"].T for bb in range(b)], axis=0)
    return np.ascontiguousarray(out, dtype=np.float32)


# revision 12
# speedup vs baseline: 1.0653x; 1.0653x over previous
"""Trainium2 Bass kernel: GatedRecurrentCell.

Math (per batch b):
    pa = x @ Wa^T + ba ; pi = x @ Wi^T + bi
    a  = sigmoid(gate) * 3**(-sigmoid(pa))
       = exp(-ln3/2 * tanh((pa+ba)/2) + (ln(sigmoid(gate)) - ln3/2))
    c  = sqrt(1-a^2) * silu(pi + bi)
    h_t = a_t*h_{t-1} + c_t   (scan over time, h_{-1}=0);  out = h

Mapping: data-parallel over batch (8 cores, 1 batch each). On-chip layout is
channels-on-partitions / time-on-free-dim so the recurrence runs natively on
the DVE `tensor_tensor_scan` instruction. The host feeds pre-transposed
operand layouts (d-major x and W for the PE's contraction-on-partitions
matmul) and transposes the [I,S] per-core result back to [S,I] on the host.

The sigmoid for the decay gate is computed as a tanh so that it lives in the
same activation-table set as Exp (sigmoid/silu/exp/sqrt are all in different
sets; a set switch costs a 1.28us table load). ACT instruction order is
pinned with add_dep_helper so same-table phases run back-to-back.
"""

import functools
import os

import numpy as np

B, S, D, I = 8, 2048, 512, 2048
P = 128
NCORES = 8
LN3 = float(np.log(3.0))

# matmul input dtype: "f32r" (full-rate fp32 mode) or "f32" (4x slower, exact)
MM_MODE = os.environ.get("GRC_MM_MODE", "f32r")
IC_GROUP = int(os.environ.get("GRC_IC_GROUP", "2"))
# free-dim tile width for elementwise work (also the PSUM supertile width)
CW = int(os.environ.get("GRC_CW", "1024"))
# which engine runs the c = q*w muls: "gpsimd" or "vector"
CMUL_ENGINE = os.environ.get("GRC_CMUL", "gpsimd")
# every Nth channel-chunk's scan runs on gpsimd (0 = all on DVE)
SCAN_GP_MOD = int(os.environ.get("GRC_SCAN_GP", "0"))


def _build_nc(s, d, i, mm_mode=MM_MODE, ic_group=IC_GROUP, cw=CW,
              cmul_engine=CMUL_ENGINE, scan_gp_mod=SCAN_GP_MOD, silu=True):
    import concourse.bacc as bacc
    import concourse.mybir as mybir
    import concourse.tile as tile
    from concourse.tile import add_dep_helper

    F32 = mybir.dt.float32
    F32R = mybir.dt.float32r
    AF = mybir.ActivationFunctionType
    ALU = mybir.AluOpType

    nd = d // P          # contraction chunks
    ni = i // P          # channel chunks (partition groups)
    cw = min(cw, s)
    nh = s // cw         # elementwise chunks per channel row
    nmm = cw // 512      # matmuls (N=512) per psum supertile
    MMDT = F32R if mm_mode == "f32r" else F32

    nc = bacc.Bacc("TRN2", target_bir_lowering=False, debug=False,
                   num_devices=NCORES)

    xT_d = nc.dram_tensor("xT", [d, s], F32, kind="ExternalInput").ap()
    waT_d = nc.dram_tensor("WaT", [ni, P, d], F32, kind="ExternalInput").ap()
    wiT_d = nc.dram_tensor("WiT", [ni, P, d], F32, kind="ExternalInput").ap()
    ba_d = nc.dram_tensor("baT", [P, ni], F32, kind="ExternalInput").ap()
    bi_d = nc.dram_tensor("biT", [P, ni], F32, kind="ExternalInput").ap()
    g_d = nc.dram_tensor("gateT", [P, ni], F32, kind="ExternalInput").ap()
    out_d = nc.dram_tensor("out", [i, s], F32, kind="ExternalOutput").ap()

    with tile.TileContext(nc) as tc:
        from contextlib import ExitStack

        with ExitStack() as ctx:
            const_pool = ctx.enter_context(tc.tile_pool(name="const", bufs=1))
            xt_pool = ctx.enter_context(tc.tile_pool(name="xt", bufs=1))
            wst_pool = ctx.enter_context(tc.tile_pool(name="wstream", bufs=1))
            ps_pool = ctx.enter_context(
                tc.tile_pool(name="mmpsum", bufs=1, space="PSUM"))
            chunk_pool = ctx.enter_context(tc.tile_pool(name="chunks", bufs=1))
            row_pool = ctx.enter_context(tc.tile_pool(name="rows", bufs=1))

            nbuf_pw = 2 * ic_group * nh // 2 + 1   # phase-wide chunk pools

            # ---- per-channel vectors -------------------------------------
            ba_t = const_pool.tile([P, ni], F32, name="ba_t")
            nc.sync.dma_start(ba_t[:], ba_d[:])
            bi_t = const_pool.tile([P, ni], F32, name="bi_t")
            nc.sync.dma_start(bi_t[:], bi_d[:])
            g_t = const_pool.tile([P, ni], F32, name="g_t")
            nc.sync.dma_start(g_t[:], g_d[:])

            act_chain = []

            def act(out_ap, in_ap, func, **kw):
                inst = nc.scalar.activation(out_ap, in_ap, func, **kw)
                if act_chain:
                    add_dep_helper(inst.ins, act_chain[-1].ins,
                                   reason="act table phase order")
                act_chain.append(inst)
                return inst

            alpha_t = const_pool.tile([P, ni], F32, name="alpha_t")
            act(alpha_t[:], g_t[:], AF.Sigmoid)
            lna_t = const_pool.tile([P, ni], F32, name="lna_t")
            act(lna_t[:], alpha_t[:], AF.Ln)
            # exp-phase bias: ln(alpha) - ln3/2 ; tanh-phase bias: ba/2
            lnam_t = const_pool.tile([P, ni], F32, name="lnam_t")
            nc.vector.tensor_scalar_add(lnam_t[:], lna_t[:], -LN3 / 2.0)
            bah_t = const_pool.tile([P, ni], F32, name="bah_t")
            nc.vector.tensor_scalar_mul(bah_t[:], ba_t[:], 0.5)

            # ---- resident x^T tiles -------------------------------------
            xT_sb = []
            for k in range(nd):
                t_ = xt_pool.tile([P, s], MMDT, name=f"xT{k}", tag=f"xT{k}")
                nc.sync.dma_start(t_[:], xT_d[k * P:(k + 1) * P, :].bitcast(MMDT))
                xT_sb.append(t_)

            def gemm(ps, w_sb, h):
                for m in range(nmm):
                    lo = h * cw + m * 512
                    for k in range(nd):
                        nc.tensor.matmul(
                            ps[:, m * 512:(m + 1) * 512],
                            w_sb[:, k * P:(k + 1) * P],
                            xT_sb[k][:, lo:lo + 512],
                            start=(k == 0), stop=(k == nd - 1))

            # ---- main loop: groups of `ic_group` channel chunks ---------
            groups = [list(range(g0, min(g0 + ic_group, ni)))
                      for g0 in range(0, ni, ic_group)]
            for ics in groups:
                # stream weights + GEMMs (PE); pi first (silu phase is first)
                pa_ps, pi_ps = {}, {}
                for ic in ics:
                    wi_sb = wst_pool.tile([P, d], MMDT, name=f"wi{ic}",
                                          tag="wi", bufs=4)
                    nc.sync.dma_start(wi_sb[:], wiT_d[ic].bitcast(MMDT))
                    wa_sb = wst_pool.tile([P, d], MMDT, name=f"wa{ic}",
                                          tag="wa", bufs=4)
                    nc.sync.dma_start(wa_sb[:], waT_d[ic].bitcast(MMDT))
                    for h in range(nh):
                        ps = ps_pool.tile([P, cw], F32, name=f"pi{ic}_{h}",
                                          tag="pi", bufs=2)
                        gemm(ps, wi_sb, h)
                        pi_ps[ic, h] = ps
                    for h in range(nh):
                        ps = ps_pool.tile([P, cw], F32, name=f"pa{ic}_{h}",
                                          tag="pa", bufs=2)
                        gemm(ps, wa_sb, h)
                        pa_ps[ic, h] = ps

                # ACT phase 1 [silu table]: w = silu(pi + bi) -------------
                w_t = {}
                for ic in ics:
                    for h in range(nh):
                        wt = chunk_pool.tile([P, cw], F32, name=f"w{ic}_{h}",
                                             tag="w", bufs=nbuf_pw)
                        if silu:
                            act(wt[:], pi_ps[ic, h][:], AF.Silu,
                                bias=bi_t[:, ic:ic + 1])
                        else:
                            # sim-compatible fallback (Silu not in CoreSim)
                            sg = chunk_pool.tile(
                                [P, cw], F32, name=f"sg{ic}_{h}",
                                tag="sg", bufs=3)
                            act(sg[:], pi_ps[ic, h][:], AF.Sigmoid,
                                bias=bi_t[:, ic:ic + 1])
                            pib = chunk_pool.tile(
                                [P, cw], F32, name=f"pib{ic}_{h}",
                                tag="pib", bufs=3)
                            act(pib[:], pi_ps[ic, h][:], AF.Identity,
                                bias=bi_t[:, ic:ic + 1])
                            nc.vector.tensor_mul(wt[:], sg[:], pib[:])
                        w_t[ic, h] = wt

                # ACT phase 2 [exp table]: t = tanh(pa/2 + ba/2) ----------
                s_t = {}
                for ic in ics:
                    for h in range(nh):
                        st = chunk_pool.tile([P, cw], F32, name=f"s{ic}_{h}",
                                             tag="s", bufs=nbuf_pw)
                        act(st[:], pa_ps[ic, h][:], AF.Tanh,
                            scale=0.5, bias=bah_t[:, ic:ic + 1])
                        s_t[ic, h] = st

                # ACT phase 3 [exp table, no reload]:
                #   a = exp(-ln3/2 * t + (ln(alpha) - ln3/2))
                a_t = {}
                for ic in ics:
                    for h in range(nh):
                        at = chunk_pool.tile([P, cw], F32, name=f"a{ic}_{h}",
                                             tag="a", bufs=nbuf_pw)
                        act(at[:], s_t[ic, h][:], AF.Exp,
                            scale=-LN3 / 2.0, bias=lnam_t[:, ic:ic + 1])
                        a_t[ic, h] = at

                # DVE: a2 = a*a (interleaves with ACT phases) -------------
                a2_t = {}
                for ic in ics:
                    for h in range(nh):
                        a2 = chunk_pool.tile([P, cw], F32, name=f"a2{ic}_{h}",
                                             tag="a2", bufs=3)
                        nc.vector.tensor_mul(a2[:], a_t[ic, h][:],
                                             a_t[ic, h][:])
                        a2_t[ic, h] = a2

                # ACT phase 4 [sqrt table]: q = sqrt(1 - a2);
                # then c = q*w on gpsimd (frees DVE for the scans)
                c_t = {}
                for ic in ics:
                    for h in range(nh):
                        q = chunk_pool.tile([P, cw], F32, name=f"q{ic}_{h}",
                                            tag="q", bufs=3)
                        act(q[:], a2_t[ic, h][:], AF.Sqrt,
                            scale=-1.0, bias=1.0)
                        cc = chunk_pool.tile([P, cw], F32, name=f"c{ic}_{h}",
                                             tag="c", bufs=5)
                        eng = (nc.gpsimd if cmul_engine == "gpsimd"
                               else nc.vector)
                        eng.tensor_mul(cc[:], q[:], w_t[ic, h][:])
                        c_t[ic, h] = cc

                # scan + DMA out ------------------------------------------
                for ic in ics:
                    hrow = row_pool.tile([P, s], F32, name=f"h{ic}", tag="h",
                                         bufs=3)
                    seng = (nc.gpsimd if scan_gp_mod and
                            (ic % scan_gp_mod == scan_gp_mod - 1)
                            else nc.vector)
                    for h in range(nh):
                        init = 0.0 if h == 0 else hrow[:, h * cw - 1:h * cw]
                        seng.tensor_tensor_scan(
                            hrow[:, h * cw:(h + 1) * cw],
                            a_t[ic, h][:], c_t[ic, h][:], init,
                            op0=ALU.mult, op1=ALU.add)
                    nc.sync.dma_start(out_d[ic * P:(ic + 1) * P, :], hrow[:])

    nc.compile()
    return nc


@functools.lru_cache(maxsize=2)
def _get_nc(s=S, d=D, i=I):
    return _build_nc(s, d, i)


LAST_RESULTS = None


def _prep_core_inputs(xb, WaT, WiT, baT, biT, gateT):
    return {"xT": np.ascontiguousarray(xb.T), "WaT": WaT, "WiT": WiT,
            "baT": baT, "biT": biT, "gateT": gateT}


def _prep_shared(Wa, ba, Wi, bi, gate, d, i):
    ni = i // P
    nd = d // P
    # WaT[ic, p, k*128+j] = Wa[ic*128+j, k*128+p]  (lhsT blocks, contiguous)
    WaT = np.ascontiguousarray(
        Wa.reshape(ni, P, nd, P).transpose(0, 3, 2, 1).reshape(ni, P, d))
    WiT = np.ascontiguousarray(
        Wi.reshape(ni, P, nd, P).transpose(0, 3, 2, 1).reshape(ni, P, d))
    baT = np.ascontiguousarray(ba.reshape(ni, P).T)
    biT = np.ascontiguousarray(bi.reshape(ni, P).T)
    gateT = np.ascontiguousarray(gate.reshape(ni, P).T)
    return WaT, WiT, baT, biT, gateT


def kernel(x, Wa, ba, Wi, bi, gate):
    global LAST_RESULTS
    from concourse.bass_utils import run_bass_kernel_spmd

    x = np.asarray(x, dtype=np.float32)
    b, s, d = x.shape
    i = Wa.shape[0]
    nc = _get_nc(s, d, i)

    WaT, WiT, baT, biT, gateT = _prep_shared(
        np.asarray(Wa, np.float32), np.asarray(ba, np.float32),
        np.asarray(Wi, np.float32), np.asarray(bi, np.float32),
        np.asarray(gate, np.float32), d, i)

    in_maps = [_prep_core_inputs(x[bb], WaT, WiT, baT, biT, gateT)
               for bb in range(b)]
    res = run_bass_kernel_spmd(nc, in_maps, list(range(b)))
    LAST_RESULTS = res
    out = np.stack([res.results[bb]["out"].T for bb in range(b)], axis=0)
    return np.ascontiguousarray(out, dtype=np.float32)


# revision 14
# speedup vs baseline: 1.1183x; 1.0497x over previous
"""Trainium2 Bass kernel: GatedRecurrentCell.

Math (per batch b):
    pa = x @ Wa^T + ba ; pi = x @ Wi^T + bi
    a  = sigmoid(gate) * 3**(-sigmoid(pa))
       = exp(-ln3/2 * tanh((pa+ba)/2) + (ln(sigmoid(gate)) - ln3/2))
    c  = sqrt(1-a^2) * silu(pi + bi)
    h_t = a_t*h_{t-1} + c_t   (scan over time, h_{-1}=0);  out = h

Mapping: data-parallel over batch (8 cores, 1 batch each). On-chip layout is
channels-on-partitions / time-on-free-dim so the recurrence runs natively on
the DVE `tensor_tensor_scan` instruction. The host feeds pre-transposed
operand layouts (d-major x and W for the PE's contraction-on-partitions
matmul) and transposes the [I,S] per-core result back to [S,I] on the host.

The sigmoid for the decay gate is computed as a tanh so that it lives in the
same activation-table set as Exp (sigmoid/silu/exp/sqrt are all in different
sets; a set switch costs a 1.28us table load). ACT instruction order is
pinned with add_dep_helper so same-table phases run back-to-back.
"""

import functools
import os

import numpy as np

B, S, D, I = 8, 2048, 512, 2048
P = 128
NCORES = 8
LN3 = float(np.log(3.0))

# matmul input dtype: "f32r" (full-rate fp32 mode) or "f32" (4x slower, exact)
MM_MODE = os.environ.get("GRC_MM_MODE", "f32r")
IC_GROUP = int(os.environ.get("GRC_IC_GROUP", "2"))
# free-dim tile width for elementwise work (also the PSUM supertile width)
CW = int(os.environ.get("GRC_CW", "1024"))
# which engine runs the c = q*w muls: "gpsimd" or "vector"
CMUL_ENGINE = os.environ.get("GRC_CMUL", "gpsimd")
# every Nth channel-chunk's scan runs on gpsimd (0 = all on DVE)
SCAN_GP_MOD = int(os.environ.get("GRC_SCAN_GP", "0"))


def _build_nc(s, d, i, mm_mode=MM_MODE, ic_group=IC_GROUP, cw=CW,
              cmul_engine=CMUL_ENGINE, scan_gp_mod=SCAN_GP_MOD, silu=True):
    import concourse.bacc as bacc
    import concourse.mybir as mybir
    import concourse.tile as tile
    from concourse.tile import add_dep_helper

    F32 = mybir.dt.float32
    F32R = mybir.dt.float32r
    AF = mybir.ActivationFunctionType
    ALU = mybir.AluOpType

    nd = d // P          # contraction chunks
    ni = i // P          # channel chunks (partition groups)
    cw = min(cw, s)
    nh = s // cw         # elementwise chunks per channel row
    nmm = cw // 512      # matmuls (N=512) per psum supertile
    MMDT = F32R if mm_mode == "f32r" else F32

    nc = bacc.Bacc("TRN2", target_bir_lowering=False, debug=False,
                   num_devices=NCORES)

    xT_d = nc.dram_tensor("xT", [d, s], F32, kind="ExternalInput").ap()
    waT_d = nc.dram_tensor("WaT", [ni, P, d], F32, kind="ExternalInput").ap()
    wiT_d = nc.dram_tensor("WiT", [ni, P, d], F32, kind="ExternalInput").ap()
    ba_d = nc.dram_tensor("baT", [P, ni], F32, kind="ExternalInput").ap()
    bi_d = nc.dram_tensor("biT", [P, ni], F32, kind="ExternalInput").ap()
    g_d = nc.dram_tensor("gateT", [P, ni], F32, kind="ExternalInput").ap()
    out_d = nc.dram_tensor("out", [i, s], F32, kind="ExternalOutput").ap()

    with tile.TileContext(nc) as tc:
        from contextlib import ExitStack

        with ExitStack() as ctx:
            const_pool = ctx.enter_context(tc.tile_pool(name="const", bufs=1))
            xt_pool = ctx.enter_context(tc.tile_pool(name="xt", bufs=1))
            wst_pool = ctx.enter_context(tc.tile_pool(name="wstream", bufs=1))
            ps_pool = ctx.enter_context(
                tc.tile_pool(name="mmpsum", bufs=1, space="PSUM"))
            chunk_pool = ctx.enter_context(tc.tile_pool(name="chunks", bufs=1))
            row_pool = ctx.enter_context(tc.tile_pool(name="rows", bufs=1))

            nbuf_pw = 2 * ic_group * nh // 2 + 1   # phase-wide chunk pools

            # ---- per-channel vectors -------------------------------------
            ba_t = const_pool.tile([P, ni], F32, name="ba_t")
            nc.sync.dma_start(ba_t[:], ba_d[:])
            bi_t = const_pool.tile([P, ni], F32, name="bi_t")
            nc.sync.dma_start(bi_t[:], bi_d[:])
            g_t = const_pool.tile([P, ni], F32, name="g_t")
            nc.sync.dma_start(g_t[:], g_d[:])

            act_chain = []

            def act(out_ap, in_ap, func, **kw):
                inst = nc.scalar.activation(out_ap, in_ap, func, **kw)
                if act_chain:
                    add_dep_helper(inst.ins, act_chain[-1].ins,
                                   reason="act table phase order")
                act_chain.append(inst)
                return inst

            alpha_t = const_pool.tile([P, ni], F32, name="alpha_t")
            act(alpha_t[:], g_t[:], AF.Sigmoid)
            lna_t = const_pool.tile([P, ni], F32, name="lna_t")
            act(lna_t[:], alpha_t[:], AF.Ln)
            # exp-phase bias: ln(alpha) - ln3/2 ; tanh-phase bias: ba/2
            lnam_t = const_pool.tile([P, ni], F32, name="lnam_t")
            nc.vector.tensor_scalar_add(lnam_t[:], lna_t[:], -LN3 / 2.0)
            bah_t = const_pool.tile([P, ni], F32, name="bah_t")
            nc.vector.tensor_scalar_mul(bah_t[:], ba_t[:], 0.5)

            # ---- resident x^T tiles -------------------------------------
            xT_sb = []
            for k in range(nd):
                t_ = xt_pool.tile([P, s], MMDT, name=f"xT{k}", tag=f"xT{k}")
                nc.sync.dma_start(t_[:], xT_d[k * P:(k + 1) * P, :].bitcast(MMDT))
                xT_sb.append(t_)

            def gemm(ps, w_sb, h):
                for m in range(nmm):
                    lo = h * cw + m * 512
                    for k in range(nd):
                        nc.tensor.matmul(
                            ps[:, m * 512:(m + 1) * 512],
                            w_sb[:, k * P:(k + 1) * P],
                            xT_sb[k][:, lo:lo + 512],
                            start=(k == 0), stop=(k == nd - 1))

            # wide grain for SBUF->SBUF elementwise stages
            ew = min(2 * cw, s)
            new = s // ew          # wide chunks per channel row

            # ---- main loop: groups of `ic_group` channel chunks ---------
            # Per group, a/wc/h live in ONE [P, len(ics)*s] buffer so the
            # recurrence runs as a single scan across all the group's
            # channels: a[channel_start] is zeroed, which exactly restarts
            # the recurrence (h0 = a0*0 + c0 never reads a0).
            groups = [list(range(g0, min(g0 + ic_group, ni)))
                      for g0 in range(0, ni, ic_group)]
            for ics in groups:
                gs = len(ics) * s      # group row length

                # stream weights + GEMMs (PE); pi first (silu phase is first)
                pa_ps, pi_ps = {}, {}
                for ic in ics:
                    wi_sb = wst_pool.tile([P, d], MMDT, name=f"wi{ic}",
                                          tag="wi", bufs=3)
                    nc.sync.dma_start(wi_sb[:], wiT_d[ic].bitcast(MMDT))
                    wa_sb = wst_pool.tile([P, d], MMDT, name=f"wa{ic}",
                                          tag="wa", bufs=3)
                    nc.sync.dma_start(wa_sb[:], waT_d[ic].bitcast(MMDT))
                    for h in range(nh):
                        ps = ps_pool.tile([P, cw], F32, name=f"pi{ic}_{h}",
                                          tag="pi", bufs=2)
                        gemm(ps, wi_sb, h)
                        pi_ps[ic, h] = ps
                    for h in range(nh):
                        ps = ps_pool.tile([P, cw], F32, name=f"pa{ic}_{h}",
                                          tag="pa", bufs=2)
                        gemm(ps, wa_sb, h)
                        pa_ps[ic, h] = ps

                wc_g = row_pool.tile([P, gs], F32, name=f"wc{ics[0]}",
                                     tag="wc", bufs=2)
                a_g = row_pool.tile([P, gs], F32, name=f"ag{ics[0]}",
                                    tag="ag", bufs=2)
                h_g = row_pool.tile([P, gs], F32, name=f"hg{ics[0]}",
                                    tag="hg", bufs=2)

                # ACT phase 1 [silu table]: w = silu(pi + bi) into wc -----
                for icg, ic in enumerate(ics):
                    for h in range(nh):
                        wt = wc_g[:, icg * s + h * cw: icg * s + (h + 1) * cw]
                        if silu:
                            act(wt, pi_ps[ic, h][:], AF.Silu,
                                bias=bi_t[:, ic:ic + 1])
                        else:
                            # sim-compatible fallback (Silu not in CoreSim)
                            sg = chunk_pool.tile(
                                [P, cw], F32, name=f"sg{ic}_{h}",
                                tag="sg", bufs=3)
                            act(sg[:], pi_ps[ic, h][:], AF.Sigmoid,
                                bias=bi_t[:, ic:ic + 1])
                            pib = chunk_pool.tile(
                                [P, cw], F32, name=f"pib{ic}_{h}",
                                tag="pib", bufs=3)
                            act(pib[:], pi_ps[ic, h][:], AF.Identity,
                                bias=bi_t[:, ic:ic + 1])
                            nc.vector.tensor_mul(wt, sg[:], pib[:])

                # ACT phase 2 [exp table]: t = tanh(pa/2 + ba/2) ----------
                s_t = {}
                for ic in ics:
                    for hw in range(new):
                        st = chunk_pool.tile([P, ew], F32, name=f"s{ic}_{hw}",
                                             tag="s", bufs=3)
                        for j in range(ew // cw):
                            act(st[:, j * cw:(j + 1) * cw],
                                pa_ps[ic, hw * (ew // cw) + j][:], AF.Tanh,
                                scale=0.5, bias=bah_t[:, ic:ic + 1])
                        s_t[ic, hw] = st

                # ACT phase 3 [exp table, no reload]:
                #   a = exp(-ln3/2 * t + (ln(alpha) - ln3/2))
                for icg, ic in enumerate(ics):
                    for hw in range(new):
                        act(a_g[:, icg * s + hw * ew: icg * s + (hw + 1) * ew],
                            s_t[ic, hw][:], AF.Exp,
                            scale=-LN3 / 2.0, bias=lnam_t[:, ic:ic + 1])
                # DVE: a2 = a*a (interleaves with ACT phases) -------------
                a2_t = {}
                for icg, ic in enumerate(ics):
                    for hw in range(new):
                        a2 = chunk_pool.tile([P, ew], F32,
                                             name=f"a2{ic}_{hw}",
                                             tag="s", bufs=3)
                        sl = a_g[:, icg * s + hw * ew: icg * s + (hw + 1) * ew]
                        nc.vector.tensor_mul(a2[:], sl, sl)
                        a2_t[ic, hw] = a2
                    if icg > 0:
                        # restart the recurrence at this channel boundary
                        # (a0 is never read by the scan: h0 = a0*0 + c0;
                        #  must happen AFTER a2 has consumed the real a0)
                        nc.gpsimd.memset(a_g[:, icg * s: icg * s + 1], 0.0)

                # ACT phase 4 [sqrt table]: q = sqrt(1 - a2);
                # then wc *= q in place (c = q*w), split DVE/gpsimd
                for icg, ic in enumerate(ics):
                    for hw in range(new):
                        q = chunk_pool.tile([P, ew], F32, name=f"q{ic}_{hw}",
                                            tag="q", bufs=3)
                        act(q[:], a2_t[ic, hw][:], AF.Sqrt,
                            scale=-1.0, bias=1.0)
                        wt = wc_g[:, icg * s + hw * ew:
                                  icg * s + (hw + 1) * ew]
                        eng = (nc.gpsimd if cmul_engine == "gpsimd"
                               else nc.vector)
                        eng.tensor_mul(wt, q[:], wt)

                # one scan across the whole group's channels --------------
                nc.vector.tensor_tensor_scan(
                    h_g[:], a_g[:], wc_g[:], 0.0,
                    op0=ALU.mult, op1=ALU.add)
                for icg, ic in enumerate(ics):
                    nc.sync.dma_start(out_d[ic * P:(ic + 1) * P, :],
                                      h_g[:, icg * s:(icg + 1) * s])

    nc.compile()
    return nc


@functools.lru_cache(maxsize=2)
def _get_nc(s=S, d=D, i=I):
    return _build_nc(s, d, i)


LAST_RESULTS = None


def _prep_core_inputs(xb, WaT, WiT, baT, biT, gateT):
    return {"xT": np.ascontiguousarray(xb.T), "WaT": WaT, "WiT": WiT,
            "baT": baT, "biT": biT, "gateT": gateT}


def _prep_shared(Wa, ba, Wi, bi, gate, d, i):
    ni = i // P
    nd = d // P
    # WaT[ic, p, k*128+j] = Wa[ic*128+j, k*128+p]  (lhsT blocks, contiguous)
    WaT = np.ascontiguousarray(
        Wa.reshape(ni, P, nd, P).transpose(0, 3, 2, 1).reshape(ni, P, d))
    WiT = np.ascontiguousarray(
        Wi.reshape(ni, P, nd, P).transpose(0, 3, 2, 1).reshape(ni, P, d))
    baT = np.ascontiguousarray(ba.reshape(ni, P).T)
    biT = np.ascontiguousarray(bi.reshape(ni, P).T)
    gateT = np.ascontiguousarray(gate.reshape(ni, P).T)
    return WaT, WiT, baT, biT, gateT


def kernel(x, Wa, ba, Wi, bi, gate):
    global LAST_RESULTS
    from concourse.bass_utils import run_bass_kernel_spmd

    x = np.asarray(x, dtype=np.float32)
    b, s, d = x.shape
    i = Wa.shape[0]
    nc = _get_nc(s, d, i)

    WaT, WiT, baT, biT, gateT = _prep_shared(
        np.asarray(Wa, np.float32), np.asarray(ba, np.float32),
        np.asarray(Wi, np.float32), np.asarray(bi, np.float32),
        np.asarray(gate, np.float32), d, i)

    in_maps = [_prep_core_inputs(x[bb], WaT, WiT, baT, biT, gateT)
               for bb in range(b)]
    res = run_bass_kernel_spmd(nc, in_maps, list(range(b)))
    LAST_RESULTS = res
    out = np.stack([res.results[bb]["out"].T for bb in range(b)], axis=0)
    return np.ascontiguousarray(out, dtype=np.float32)


# revision 16
# speedup vs baseline: 1.1450x; 1.0238x over previous
"""Trainium2 Bass kernel: GatedRecurrentCell.

Math (per batch b):
    pa = x @ Wa^T + ba ; pi = x @ Wi^T + bi
    a  = sigmoid(gate) * 3**(-sigmoid(pa))
       = exp(-ln3/2 * tanh((pa+ba)/2) + (ln(sigmoid(gate)) - ln3/2))
    c  = sqrt(1-a^2) * silu(pi + bi)
    h_t = a_t*h_{t-1} + c_t   (scan over time, h_{-1}=0);  out = h

Mapping: data-parallel over batch (8 cores, 1 batch each). On-chip layout is
channels-on-partitions / time-on-free-dim so the recurrence runs natively on
the DVE `tensor_tensor_scan` instruction. The host feeds pre-transposed
operand layouts (d-major x and W for the PE's contraction-on-partitions
matmul) and transposes the [I,S] per-core result back to [S,I] on the host.

The sigmoid for the decay gate is computed as a tanh so that it lives in the
same activation-table set as Exp (sigmoid/silu/exp/sqrt are all in different
sets; a set switch costs a 1.28us table load). ACT instruction order is
pinned with add_dep_helper so same-table phases run back-to-back.
"""

import functools
import os

import numpy as np

B, S, D, I = 8, 2048, 512, 2048
P = 128
NCORES = 8
LN3 = float(np.log(3.0))

# matmul input dtype: "f32r" (full-rate fp32 mode) or "f32" (4x slower, exact)
MM_MODE = os.environ.get("GRC_MM_MODE", "f32r")
IC_GROUP = int(os.environ.get("GRC_IC_GROUP", "2"))
# free-dim tile width for elementwise work (also the PSUM supertile width)
CW = int(os.environ.get("GRC_CW", "1024"))
# which engine runs the c = q*w muls: "gpsimd" or "vector"
CMUL_ENGINE = os.environ.get("GRC_CMUL", "gpsimd")
# every Nth channel-chunk's scan runs on gpsimd (0 = all on DVE)
SCAN_GP_MOD = int(os.environ.get("GRC_SCAN_GP", "0"))


def _build_nc(s, d, i, mm_mode=MM_MODE, ic_group=IC_GROUP, cw=CW,
              cmul_engine=CMUL_ENGINE, scan_gp_mod=SCAN_GP_MOD, silu=True):
    import concourse.bacc as bacc
    import concourse.mybir as mybir
    import concourse.tile as tile
    from concourse.tile import add_dep_helper

    F32 = mybir.dt.float32
    F32R = mybir.dt.float32r
    AF = mybir.ActivationFunctionType
    ALU = mybir.AluOpType

    nd = d // P          # contraction chunks
    ni = i // P          # channel chunks (partition groups)
    cw = min(cw, s)
    nh = s // cw         # elementwise chunks per channel row
    nmm = cw // 512      # matmuls (N=512) per psum supertile
    MMDT = F32R if mm_mode == "f32r" else F32

    nc = bacc.Bacc("TRN2", target_bir_lowering=False, debug=False,
                   num_devices=NCORES)

    xT_d = nc.dram_tensor("xT", [d, s], F32, kind="ExternalInput").ap()
    waT_d = nc.dram_tensor("WaT", [ni, P, d], F32, kind="ExternalInput").ap()
    wiT_d = nc.dram_tensor("WiT", [ni, P, d], F32, kind="ExternalInput").ap()
    ba_d = nc.dram_tensor("baT", [P, ni], F32, kind="ExternalInput").ap()
    bi_d = nc.dram_tensor("biT", [P, ni], F32, kind="ExternalInput").ap()
    g_d = nc.dram_tensor("gateT", [P, ni], F32, kind="ExternalInput").ap()
    out_d = nc.dram_tensor("out", [i, s], F32, kind="ExternalOutput").ap()

    with tile.TileContext(nc) as tc:
        from contextlib import ExitStack

        with ExitStack() as ctx:
            const_pool = ctx.enter_context(tc.tile_pool(name="const", bufs=1))
            xt_pool = ctx.enter_context(tc.tile_pool(name="xt", bufs=1))
            wst_pool = ctx.enter_context(tc.tile_pool(name="wstream", bufs=1))
            ps_pool = ctx.enter_context(
                tc.tile_pool(name="mmpsum", bufs=1, space="PSUM"))
            chunk_pool = ctx.enter_context(tc.tile_pool(name="chunks", bufs=1))
            row_pool = ctx.enter_context(tc.tile_pool(name="rows", bufs=1))

            nbuf_pw = 2 * ic_group * nh // 2 + 1   # phase-wide chunk pools

            # ---- per-channel vectors -------------------------------------
            ba_t = const_pool.tile([P, ni], F32, name="ba_t")
            nc.sync.dma_start(ba_t[:], ba_d[:])
            bi_t = const_pool.tile([P, ni], F32, name="bi_t")
            nc.sync.dma_start(bi_t[:], bi_d[:])
            g_t = const_pool.tile([P, ni], F32, name="g_t")
            nc.sync.dma_start(g_t[:], g_d[:])

            act_chain = []

            def act(out_ap, in_ap, func, **kw):
                inst = nc.scalar.activation(out_ap, in_ap, func, **kw)
                if act_chain:
                    add_dep_helper(inst.ins, act_chain[-1].ins, False,
                                   "act table phase order")
                act_chain.append(inst)
                return inst

            alpha_t = const_pool.tile([P, ni], F32, name="alpha_t")
            act(alpha_t[:], g_t[:], AF.Sigmoid)
            lna_t = const_pool.tile([P, ni], F32, name="lna_t")
            act(lna_t[:], alpha_t[:], AF.Ln)
            # exp-phase bias: ln(alpha) - ln3/2 ; tanh-phase bias: ba/2
            lnam_t = const_pool.tile([P, ni], F32, name="lnam_t")
            nc.vector.tensor_scalar_add(lnam_t[:], lna_t[:], -LN3 / 2.0)
            bah_t = const_pool.tile([P, ni], F32, name="bah_t")
            nc.vector.tensor_scalar_mul(bah_t[:], ba_t[:], 0.5)

            # ---- resident x^T tiles -------------------------------------
            xT_sb = []
            for k in range(nd):
                t_ = xt_pool.tile([P, s], MMDT, name=f"xT{k}", tag=f"xT{k}")
                nc.sync.dma_start(t_[:], xT_d[k * P:(k + 1) * P, :].bitcast(MMDT))
                xT_sb.append(t_)

            def gemm(ps, w_sb, h):
                for m in range(nmm):
                    lo = h * cw + m * 512
                    for k in range(nd):
                        nc.tensor.matmul(
                            ps[:, m * 512:(m + 1) * 512],
                            w_sb[:, k * P:(k + 1) * P],
                            xT_sb[k][:, lo:lo + 512],
                            start=(k == 0), stop=(k == nd - 1))

            # wide grain for SBUF->SBUF elementwise stages
            ew = min(2 * cw, s)
            new = s // ew          # wide chunks per channel row

            # ---- main loop: groups of `ic_group` channel chunks ---------
            # Per group, a/wc/h live in ONE [P, len(ics)*s] buffer so the
            # recurrence runs as a single scan across all the group's
            # channels: a[channel_start] is zeroed, which exactly restarts
            # the recurrence (h0 = a0*0 + c0 never reads a0).
            groups = [list(range(g0, min(g0 + ic_group, ni)))
                      for g0 in range(0, ni, ic_group)]
            for ics in groups:
                gs = len(ics) * s      # group row length

                # stream weights + GEMMs (PE); all pi GEMMs for the group
                # first (the silu phase consumes them first), then all pa.
                pa_ps, pi_ps = {}, {}
                wa_sbs = {}
                for ic in ics:
                    wi_sb = wst_pool.tile([P, d], MMDT, name=f"wi{ic}",
                                          tag="wi", bufs=3)
                    nc.sync.dma_start(wi_sb[:], wiT_d[ic].bitcast(MMDT))
                    wa_sb = wst_pool.tile([P, d], MMDT, name=f"wa{ic}",
                                          tag="wa", bufs=3)
                    nc.sync.dma_start(wa_sb[:], waT_d[ic].bitcast(MMDT))
                    wa_sbs[ic] = wa_sb
                    for h in range(nh):
                        ps = ps_pool.tile([P, cw], F32, name=f"pi{ic}_{h}",
                                          tag="pi", bufs=2)
                        gemm(ps, wi_sb, h)
                        pi_ps[ic, h] = ps
                for ic in ics:
                    for h in range(nh):
                        ps = ps_pool.tile([P, cw], F32, name=f"pa{ic}_{h}",
                                          tag="pa", bufs=2)
                        gemm(ps, wa_sbs[ic], h)
                        pa_ps[ic, h] = ps

                wc_g = row_pool.tile([P, gs], F32, name=f"wc{ics[0]}",
                                     tag="wc", bufs=2)
                a_g = row_pool.tile([P, gs], F32, name=f"ag{ics[0]}",
                                    tag="ag", bufs=2)
                h_g = row_pool.tile([P, gs], F32, name=f"hg{ics[0]}",
                                    tag="hg", bufs=2)

                # ACT phase 1 [silu table]: w = silu(pi + bi) into wc -----
                for icg, ic in enumerate(ics):
                    for h in range(nh):
                        wt = wc_g[:, icg * s + h * cw: icg * s + (h + 1) * cw]
                        if silu:
                            act(wt, pi_ps[ic, h][:], AF.Silu,
                                bias=bi_t[:, ic:ic + 1])
                        else:
                            # sim-compatible fallback (Silu not in CoreSim)
                            sg = chunk_pool.tile(
                                [P, cw], F32, name=f"sg{ic}_{h}",
                                tag="sg", bufs=3)
                            act(sg[:], pi_ps[ic, h][:], AF.Sigmoid,
                                bias=bi_t[:, ic:ic + 1])
                            pib = chunk_pool.tile(
                                [P, cw], F32, name=f"pib{ic}_{h}",
                                tag="pib", bufs=3)
                            act(pib[:], pi_ps[ic, h][:], AF.Identity,
                                bias=bi_t[:, ic:ic + 1])
                            nc.vector.tensor_mul(wt, sg[:], pib[:])

                # ACT phase 2 [exp table]: t = tanh(pa/2 + ba/2) ----------
                s_t = {}
                for ic in ics:
                    for hw in range(new):
                        st = chunk_pool.tile([P, ew], F32, name=f"s{ic}_{hw}",
                                             tag="s", bufs=3)
                        for j in range(ew // cw):
                            act(st[:, j * cw:(j + 1) * cw],
                                pa_ps[ic, hw * (ew // cw) + j][:], AF.Tanh,
                                scale=0.5, bias=bah_t[:, ic:ic + 1])
                        s_t[ic, hw] = st

                # ACT phase 3 [exp table, no reload]:
                #   a = exp(-ln3/2 * t + (ln(alpha) - ln3/2))
                for icg, ic in enumerate(ics):
                    for hw in range(new):
                        act(a_g[:, icg * s + hw * ew: icg * s + (hw + 1) * ew],
                            s_t[ic, hw][:], AF.Exp,
                            scale=-LN3 / 2.0, bias=lnam_t[:, ic:ic + 1])
                # DVE: a2 = a*a (interleaves with ACT phases) -------------
                a2_t = {}
                for icg, ic in enumerate(ics):
                    for hw in range(new):
                        a2 = chunk_pool.tile([P, ew], F32,
                                             name=f"a2{ic}_{hw}",
                                             tag="s", bufs=3)
                        sl = a_g[:, icg * s + hw * ew: icg * s + (hw + 1) * ew]
                        nc.vector.tensor_mul(a2[:], sl, sl)
                        a2_t[ic, hw] = a2
                    if icg > 0:
                        # restart the recurrence at this channel boundary
                        # (a0 is never read by the scan: h0 = a0*0 + c0;
                        #  must happen AFTER a2 has consumed the real a0)
                        nc.gpsimd.memset(a_g[:, icg * s: icg * s + 1], 0.0)

                # ACT phase 4 [sqrt table]: q = sqrt(1 - a2);
                # then wc *= q in place (c = q*w), split DVE/gpsimd
                for icg, ic in enumerate(ics):
                    for hw in range(new):
                        q = chunk_pool.tile([P, ew], F32, name=f"q{ic}_{hw}",
                                            tag="q", bufs=3)
                        act(q[:], a2_t[ic, hw][:], AF.Sqrt,
                            scale=-1.0, bias=1.0)
                        wt = wc_g[:, icg * s + hw * ew:
                                  icg * s + (hw + 1) * ew]
                        eng = (nc.gpsimd if cmul_engine == "gpsimd"
                               else nc.vector)
                        eng.tensor_mul(wt, q[:], wt)

                # one scan across the whole group's channels --------------
                nc.vector.tensor_tensor_scan(
                    h_g[:], a_g[:], wc_g[:], 0.0,
                    op0=ALU.mult, op1=ALU.add)
                for icg, ic in enumerate(ics):
                    nc.sync.dma_start(out_d[ic * P:(ic + 1) * P, :],
                                      h_g[:, icg * s:(icg + 1) * s])

    nc.compile()
    return nc


@functools.lru_cache(maxsize=2)
def _get_nc(s=S, d=D, i=I):
    return _build_nc(s, d, i)


LAST_RESULTS = None


def _prep_core_inputs(xb, WaT, WiT, baT, biT, gateT):
    return {"xT": np.ascontiguousarray(xb.T), "WaT": WaT, "WiT": WiT,
            "baT": baT, "biT": biT, "gateT": gateT}


def _prep_shared(Wa, ba, Wi, bi, gate, d, i):
    ni = i // P
    nd = d // P
    # WaT[ic, p, k*128+j] = Wa[ic*128+j, k*128+p]  (lhsT blocks, contiguous)
    WaT = np.ascontiguousarray(
        Wa.reshape(ni, P, nd, P).transpose(0, 3, 2, 1).reshape(ni, P, d))
    WiT = np.ascontiguousarray(
        Wi.reshape(ni, P, nd, P).transpose(0, 3, 2, 1).reshape(ni, P, d))
    baT = np.ascontiguousarray(ba.reshape(ni, P).T)
    biT = np.ascontiguousarray(bi.reshape(ni, P).T)
    gateT = np.ascontiguousarray(gate.reshape(ni, P).T)
    return WaT, WiT, baT, biT, gateT


def kernel(x, Wa, ba, Wi, bi, gate):
    global LAST_RESULTS
    from concourse.bass_utils import run_bass_kernel_spmd

    x = np.asarray(x, dtype=np.float32)
    b, s, d = x.shape
    i = Wa.shape[0]
    nc = _get_nc(s, d, i)

    WaT, WiT, baT, biT, gateT = _prep_shared(
        np.asarray(Wa, np.float32), np.asarray(ba, np.float32),
        np.asarray(Wi, np.float32), np.asarray(bi, np.float32),
        np.asarray(gate, np.float32), d, i)

    in_maps = [_prep_core_inputs(x[bb], WaT, WiT, baT, biT, gateT)
               for bb in range(b)]
    res = run_bass_kernel_spmd(nc, in_maps, list(range(b)))
    LAST_RESULTS = res
    out = np.stack([res.results[bb]["out"].T for bb in range(b)], axis=0)
    return np.ascontiguousarray(out, dtype=np.float32)


# revision 19
# speedup vs baseline: 1.1771x; 1.0281x over previous
"""Trainium2 Bass kernel: GatedRecurrentCell.

Math (per batch b):
    pa = x @ Wa^T + ba ; pi = x @ Wi^T + bi
    a  = sigmoid(gate) * 3**(-sigmoid(pa))
       = exp(-ln3/2 * tanh((pa+ba)/2) + (ln(sigmoid(gate)) - ln3/2))
    c  = sqrt(1-a^2) * silu(pi + bi)
    h_t = a_t*h_{t-1} + c_t   (scan over time, h_{-1}=0);  out = h

Mapping: data-parallel over batch (8 cores, 1 batch each). On-chip layout is
channels-on-partitions / time-on-free-dim so the recurrence runs natively on
the DVE `tensor_tensor_scan` instruction. The host feeds pre-transposed
operand layouts (d-major x and W for the PE's contraction-on-partitions
matmul) and transposes the [I,S] per-core result back to [S,I] on the host.

The sigmoid for the decay gate is computed as a tanh so that it lives in the
same activation-table set as Exp (sigmoid/silu/exp/sqrt are all in different
sets; a set switch costs a 1.28us table load). ACT instruction order is
pinned with add_dep_helper so same-table phases run back-to-back.
"""

import functools
import os

import numpy as np

B, S, D, I = 8, 2048, 512, 2048
P = 128
NCORES = 8
LN3 = float(np.log(3.0))

# matmul input dtype: "f32r" (full-rate fp32 mode) or "f32" (4x slower, exact)
MM_MODE = os.environ.get("GRC_MM_MODE", "f32r")
IC_GROUP = int(os.environ.get("GRC_IC_GROUP", "2"))
# free-dim tile width for elementwise work (also the PSUM supertile width)
CW = int(os.environ.get("GRC_CW", "1024"))
# which engine runs the c = q*w muls: "gpsimd" or "vector"
CMUL_ENGINE = os.environ.get("GRC_CMUL", "gpsimd")
# every Nth channel-chunk's scan runs on gpsimd (0 = all on DVE)
SCAN_GP_MOD = int(os.environ.get("GRC_SCAN_GP", "0"))


def _build_nc(s, d, i, mm_mode=MM_MODE, ic_group=IC_GROUP, cw=CW,
              cmul_engine=CMUL_ENGINE, scan_gp_mod=SCAN_GP_MOD, silu=True):
    import concourse.bacc as bacc
    import concourse.mybir as mybir
    import concourse.tile as tile
    from concourse.tile import add_dep_helper

    F32 = mybir.dt.float32
    F32R = mybir.dt.float32r
    AF = mybir.ActivationFunctionType
    ALU = mybir.AluOpType

    nd = d // P          # contraction chunks
    ni = i // P          # channel chunks (partition groups)
    cw = min(cw, s)
    nh = s // cw         # elementwise chunks per channel row
    nmm = cw // 512      # matmuls (N=512) per psum supertile
    MMDT = F32R if mm_mode == "f32r" else F32

    nc = bacc.Bacc("TRN2", target_bir_lowering=False, debug=False,
                   num_devices=NCORES)

    xT_d = nc.dram_tensor("xT", [d, s], F32, kind="ExternalInput").ap()
    waT_d = nc.dram_tensor("WaT", [ni, P, d], F32, kind="ExternalInput").ap()
    wiT_d = nc.dram_tensor("WiT", [ni, P, d], F32, kind="ExternalInput").ap()
    ba_d = nc.dram_tensor("baT", [P, ni], F32, kind="ExternalInput").ap()
    bi_d = nc.dram_tensor("biT", [P, ni], F32, kind="ExternalInput").ap()
    g_d = nc.dram_tensor("gateT", [P, ni], F32, kind="ExternalInput").ap()
    out_d = nc.dram_tensor("out", [i, s], F32, kind="ExternalOutput").ap()

    with tile.TileContext(nc) as tc:
        from contextlib import ExitStack

        with ExitStack() as ctx:
            const_pool = ctx.enter_context(tc.tile_pool(name="const", bufs=1))
            xt_pool = ctx.enter_context(tc.tile_pool(name="xt", bufs=1))
            wst_pool = ctx.enter_context(tc.tile_pool(name="wstream", bufs=1))
            ps_pool = ctx.enter_context(
                tc.tile_pool(name="mmpsum", bufs=1, space="PSUM"))
            chunk_pool = ctx.enter_context(tc.tile_pool(name="chunks", bufs=1))
            row_pool = ctx.enter_context(tc.tile_pool(name="rows", bufs=1))

            nbuf_pw = 2 * ic_group * nh // 2 + 1   # phase-wide chunk pools

            # ---- per-channel vectors -------------------------------------
            ba_t = const_pool.tile([P, ni], F32, name="ba_t")
            nc.sync.dma_start(ba_t[:], ba_d[:])
            bi_t = const_pool.tile([P, ni], F32, name="bi_t")
            nc.sync.dma_start(bi_t[:], bi_d[:])
            g_t = const_pool.tile([P, ni], F32, name="g_t")
            nc.sync.dma_start(g_t[:], g_d[:])

            act_chain = []

            def act(out_ap, in_ap, func, **kw):
                inst = nc.scalar.activation(out_ap, in_ap, func, **kw)
                if act_chain:
                    add_dep_helper(inst.ins, act_chain[-1].ins, False,
                                   "act table phase order")
                act_chain.append(inst)
                return inst

            alpha_t = const_pool.tile([P, ni], F32, name="alpha_t")
            act(alpha_t[:], g_t[:], AF.Sigmoid)
            lna_t = const_pool.tile([P, ni], F32, name="lna_t")
            act(lna_t[:], alpha_t[:], AF.Ln)
            # exp-phase bias: ln(alpha) - ln3/2 ; tanh-phase bias: ba/2
            lnam_t = const_pool.tile([P, ni], F32, name="lnam_t")
            nc.vector.tensor_scalar_add(lnam_t[:], lna_t[:], -LN3 / 2.0)
            bah_t = const_pool.tile([P, ni], F32, name="bah_t")
            nc.vector.tensor_scalar_mul(bah_t[:], ba_t[:], 0.5)

            # ---- resident x^T tiles -------------------------------------
            xT_sb = []
            for k in range(nd):
                t_ = xt_pool.tile([P, s], MMDT, name=f"xT{k}", tag=f"xT{k}")
                xT_sb.append(t_)
            # column-chunked, k-interleaved loads so the first GEMM's
            # operands (all k, first columns) arrive as early as possible
            for h in range(nh):
                for k in range(nd):
                    nc.sync.dma_start(
                        xT_sb[k][:, h * cw:(h + 1) * cw],
                        xT_d[k * P:(k + 1) * P,
                             h * cw:(h + 1) * cw].bitcast(MMDT))

            def gemm(ps, w_sb, h):
                for m in range(nmm):
                    lo = h * cw + m * 512
                    for k in range(nd):
                        nc.tensor.matmul(
                            ps[:, m * 512:(m + 1) * 512],
                            w_sb[:, k * P:(k + 1) * P],
                            xT_sb[k][:, lo:lo + 512],
                            start=(k == 0), stop=(k == nd - 1))

            # wide grain for SBUF->SBUF elementwise stages
            ew = min(2 * cw, s)
            new = s // ew          # wide chunks per channel row

            # ---- main loop: groups of `ic_group` channel chunks ---------
            # Per group, a/wc/h live in ONE [P, len(ics)*s] buffer so the
            # recurrence runs as a single scan across all the group's
            # channels: a[channel_start] is zeroed, which exactly restarts
            # the recurrence (h0 = a0*0 + c0 never reads a0).
            groups = [list(range(g0, min(g0 + ic_group, ni)))
                      for g0 in range(0, ni, ic_group)]
            if ic_group > 1 and ni > 2:
                # split the final group into singletons: the kernel tail is
                # the last group's (c-mul -> scan -> DMA) chain, so keep it
                # short and run its muls on the faster DVE
                last = groups.pop()
                groups.extend([ic] for ic in last)
            for ics in groups:
                is_tail = len(ics) == 1
                gs = len(ics) * s      # group row length

                # stream weights + GEMMs (PE); all pi GEMMs for the group
                # first (the silu phase consumes them first), then all pa.
                pa_ps, pi_ps = {}, {}
                wa_sbs = {}
                for ic in ics:
                    wi_sb = wst_pool.tile([P, d], MMDT, name=f"wi{ic}",
                                          tag="wi", bufs=3)
                    nc.sync.dma_start(wi_sb[:], wiT_d[ic].bitcast(MMDT))
                    wa_sb = wst_pool.tile([P, d], MMDT, name=f"wa{ic}",
                                          tag="wa", bufs=3)
                    nc.sync.dma_start(wa_sb[:], waT_d[ic].bitcast(MMDT))
                    wa_sbs[ic] = wa_sb
                    for h in range(nh):
                        ps = ps_pool.tile([P, cw], F32, name=f"pi{ic}_{h}",
                                          tag="pi", bufs=2)
                        gemm(ps, wi_sb, h)
                        pi_ps[ic, h] = ps
                for ic in ics:
                    for h in range(nh):
                        ps = ps_pool.tile([P, cw], F32, name=f"pa{ic}_{h}",
                                          tag="pa", bufs=2)
                        gemm(ps, wa_sbs[ic], h)
                        pa_ps[ic, h] = ps

                wc_g = row_pool.tile([P, gs], F32, name=f"wc{ics[0]}",
                                     tag="wc", bufs=2)
                a_g = row_pool.tile([P, gs], F32, name=f"ag{ics[0]}",
                                    tag="ag", bufs=2)
                h_g = row_pool.tile([P, gs], F32, name=f"hg{ics[0]}",
                                    tag="hg", bufs=2)

                # ACT phase 1 [silu table]: w = silu(pi + bi) into wc -----
                for icg, ic in enumerate(ics):
                    for h in range(nh):
                        wt = wc_g[:, icg * s + h * cw: icg * s + (h + 1) * cw]
                        if silu:
                            act(wt, pi_ps[ic, h][:], AF.Silu,
                                bias=bi_t[:, ic:ic + 1])
                        else:
                            # sim-compatible fallback (Silu not in CoreSim)
                            sg = chunk_pool.tile(
                                [P, cw], F32, name=f"sg{ic}_{h}",
                                tag="sg", bufs=3)
                            act(sg[:], pi_ps[ic, h][:], AF.Sigmoid,
                                bias=bi_t[:, ic:ic + 1])
                            pib = chunk_pool.tile(
                                [P, cw], F32, name=f"pib{ic}_{h}",
                                tag="pib", bufs=3)
                            act(pib[:], pi_ps[ic, h][:], AF.Identity,
                                bias=bi_t[:, ic:ic + 1])
                            nc.vector.tensor_mul(wt, sg[:], pib[:])

                # ACT phase 2 [exp table]: t = tanh(pa/2 + ba/2) ----------
                s_t = {}
                for ic in ics:
                    for hw in range(new):
                        st = chunk_pool.tile([P, ew], F32, name=f"s{ic}_{hw}",
                                             tag="s", bufs=3)
                        for j in range(ew // cw):
                            act(st[:, j * cw:(j + 1) * cw],
                                pa_ps[ic, hw * (ew // cw) + j][:], AF.Tanh,
                                scale=0.5, bias=bah_t[:, ic:ic + 1])
                        s_t[ic, hw] = st

                # ACT phase 3 [exp table, no reload]:
                #   a = exp(-ln3/2 * t + (ln(alpha) - ln3/2))
                for icg, ic in enumerate(ics):
                    for hw in range(new):
                        act(a_g[:, icg * s + hw * ew: icg * s + (hw + 1) * ew],
                            s_t[ic, hw][:], AF.Exp,
                            scale=-LN3 / 2.0, bias=lnam_t[:, ic:ic + 1])
                # DVE: a2 = a*a (interleaves with ACT phases) -------------
                a2_t = {}
                for icg, ic in enumerate(ics):
                    for hw in range(new):
                        a2 = chunk_pool.tile([P, ew], F32,
                                             name=f"a2{ic}_{hw}",
                                             tag="s", bufs=3)
                        sl = a_g[:, icg * s + hw * ew: icg * s + (hw + 1) * ew]
                        nc.vector.tensor_mul(a2[:], sl, sl)
                        a2_t[ic, hw] = a2
                    if icg > 0:
                        # restart the recurrence at this channel boundary
                        # (a0 is never read by the scan: h0 = a0*0 + c0;
                        #  must happen AFTER a2 has consumed the real a0)
                        nc.gpsimd.memset(a_g[:, icg * s: icg * s + 1], 0.0)

                # ACT phase 4 [sqrt table]: q = sqrt(1 - a2);
                # then wc *= q in place (c = q*w), split DVE/gpsimd
                for icg, ic in enumerate(ics):
                    for hw in range(new):
                        q = chunk_pool.tile([P, ew], F32, name=f"q{ic}_{hw}",
                                            tag="q", bufs=3)
                        act(q[:], a2_t[ic, hw][:], AF.Sqrt,
                            scale=-1.0, bias=1.0)
                        wt = wc_g[:, icg * s + hw * ew:
                                  icg * s + (hw + 1) * ew]
                        eng = (nc.gpsimd
                               if cmul_engine == "gpsimd" and not is_tail
                               else nc.vector)
                        eng.tensor_mul(wt, q[:], wt)

                # one scan across the whole group's channels --------------
                nc.vector.tensor_tensor_scan(
                    h_g[:], a_g[:], wc_g[:], 0.0,
                    op0=ALU.mult, op1=ALU.add)
                for icg, ic in enumerate(ics):
                    nc.sync.dma_start(out_d[ic * P:(ic + 1) * P, :],
                                      h_g[:, icg * s:(icg + 1) * s])

    nc.compile()
    return nc


@functools.lru_cache(maxsize=2)
def _get_nc(s=S, d=D, i=I):
    return _build_nc(s, d, i)


LAST_RESULTS = None


def _prep_core_inputs(xb, WaT, WiT, baT, biT, gateT):
    return {"xT": np.ascontiguousarray(xb.T), "WaT": WaT, "WiT": WiT,
            "baT": baT, "biT": biT, "gateT": gateT}


def _prep_shared(Wa, ba, Wi, bi, gate, d, i):
    ni = i // P
    nd = d // P
    # WaT[ic, p, k*128+j] = Wa[ic*128+j, k*128+p]  (lhsT blocks, contiguous)
    WaT = np.ascontiguousarray(
        Wa.reshape(ni, P, nd, P).transpose(0, 3, 2, 1).reshape(ni, P, d))
    WiT = np.ascontiguousarray(
        Wi.reshape(ni, P, nd, P).transpose(0, 3, 2, 1).reshape(ni, P, d))
    baT = np.ascontiguousarray(ba.reshape(ni, P).T)
    biT = np.ascontiguousarray(bi.reshape(ni, P).T)
    gateT = np.ascontiguousarray(gate.reshape(ni, P).T)
    return WaT, WiT, baT, biT, gateT


def kernel(x, Wa, ba, Wi, bi, gate):
    global LAST_RESULTS
    from concourse.bass_utils import run_bass_kernel_spmd

    x = np.asarray(x, dtype=np.float32)
    b, s, d = x.shape
    i = Wa.shape[0]
    nc = _get_nc(s, d, i)

    WaT, WiT, baT, biT, gateT = _prep_shared(
        np.asarray(Wa, np.float32), np.asarray(ba, np.float32),
        np.asarray(Wi, np.float32), np.asarray(bi, np.float32),
        np.asarray(gate, np.float32), d, i)

    in_maps = [_prep_core_inputs(x[bb], WaT, WiT, baT, biT, gateT)
               for bb in range(b)]
    res = run_bass_kernel_spmd(nc, in_maps, list(range(b)))
    LAST_RESULTS = res
    out = np.stack([res.results[bb]["out"].T for bb in range(b)], axis=0)
    return np.ascontiguousarray(out, dtype=np.float32)
